# revision 1
# baseline (speedup 1.0000x reference)
"""Trainium2 Bass kernel for single-head attention (nn_MultiHeadAttention).

Reference computation (B=4, S=2048, D=1024, fp32):
    K = _K @ Wk.T + bk ; V = _V @ Wv.T + bv ; Q = _Q @ Wq.T + bq
    scores[b,k,q] = (K[b,k,:] . Q[b,q,:]) / sqrt(D)
    alpha = softmax(scores, axis=keys)
    V_[b,q,:] = sum_k V[b,k,:] * alpha[b,k,q]
    O = V_ @ Wo.T + bo

Sharding: core c = (b, h) with b = c//2 (batch), h = c%2 (query half of
1024). Each core handles the full key/value sequence of its batch and a
1024-query slice — fully data-parallel, no collectives.

Device-side layout strategy (per core):
  - Host pre-transposes activations/weights so every matmul contraction
    dim lands on SBUF partitions: _K.T/_V.T/_Q.T as [d, s], W.T as [d, e].
  - Projections produce K.T and Q.T as [e, s] (feature on partitions) and
    V naturally as [k, e]; scores = K.T' @ Q.T gives [k, q] tiles.
  - Softmax over keys (the partition dim) avoids a partition reduction:
    exp(scores/32) is taken unstabilized (scores ~ N(0,1), max << 88) and
    the key-sums are computed with an all-ones stationary matmul, which
    broadcasts sum_k es[k,q] across all 128 partitions.
  - Normalization is deferred: unnormalized V.T@es = [e, q] tiles are
    scaled by 1/sum (free-dim aligned thanks to the broadcast trick), then
    the output projection consumes them as stationary operands.
All matmuls are bf16 (M=128, N=512) accumulating in fp32 PSUM.
"""

import sys

if "/opt/trn_rl_repo" not in sys.path:
    sys.path.insert(0, "/opt/trn_rl_repo")

import ml_dtypes
import numpy as np

import concourse.bass as bass
import concourse.tile as tile
from concourse import bacc, mybir
from concourse.bass_utils import run_bass_kernel_spmd

B, S, D = 4, 2048, 1024
SQ = 1024  # queries per core
SH = 1024  # keys projected per core (half of S; pair AllGather fills the rest)
P = 128  # partitions
CH = 512  # matmul moving free dim (one fp32 PSUM bank)
EB = D // P  # 8 feature blocks
DB = D // P  # 8 contraction blocks
KB = S // P  # 16 key blocks
QB = SQ // P  # 8 query blocks
KC = S // CH  # 4 key chunks
QC = SQ // CH  # 2 query chunks
FC = D // CH  # 2 output-feature chunks
SCALE = 1.0 / np.sqrt(np.float32(D))  # folded into exp()

F32 = mybir.dt.float32
BF16 = mybir.dt.bfloat16
AF = mybir.ActivationFunctionType
NPBF16 = ml_dtypes.bfloat16

# test.py can flip this to get a profiled run; the measured NEFF time (max
# over traced cores) lands in LAST_EXEC_NS.
TRACE = False
TRACE_ALL_CORES = False
LAST_EXEC_NS = None

_NC_CACHE = None


def _build_nc() -> bass.Bass:
    # Bacc (not plain Bass): its finalize() pipeline splits multi-sem waits
    # into event-semaphore chains — TRN2 instructions take at most 1 wait.
    nc = bacc.Bacc(num_devices=8)

    kt_d = nc.dram_tensor("kt", [D, SH], BF16, kind="ExternalInput")
    vt_d = nc.dram_tensor("vt", [D, SH], BF16, kind="ExternalInput")
    qt_d = nc.dram_tensor("qt", [D, SQ], BF16, kind="ExternalInput")
    wkt_d = nc.dram_tensor("wkt", [D, D], BF16, kind="ExternalInput")
    wqt_d = nc.dram_tensor("wqt", [D, D], BF16, kind="ExternalInput")
    wvt_d = nc.dram_tensor("wvt", [D, D], BF16, kind="ExternalInput")
    wot_d = nc.dram_tensor("wot", [D, D], BF16, kind="ExternalInput")
    bk_d = nc.dram_tensor("bk", [P, EB], F32, kind="ExternalInput")
    bq_d = nc.dram_tensor("bq", [P, EB], F32, kind="ExternalInput")
    bvb_d = nc.dram_tensor("bvb", [P, D], F32, kind="ExternalInput")
    bob_d = nc.dram_tensor("bob", [P, D], F32, kind="ExternalInput")
    o_d = nc.dram_tensor("o", [SQ, D], F32, kind="ExternalOutput")

    with tile.TileContext(nc) as tc:
        # Pools are stack-allocated per SBUF side. Layout rule: regions that
        # DMA ever lands in (weights, input streams) are never reused by a
        # later pool — a fresh tile in a DMA-recycled region would carry a
        # WAR wait on every HW DMA queue and blow the per-instruction sync
        # wait-table limit (8) in walrus. Only wa (released, region then
        # left dead) and kqt (ACT-written only, safely recycled for vtu/o)
        # are ever released mid-kernel.
        p_misc = tc.alloc_tile_pool(name="misc", bufs=1, side="left")
        p_wo = tc.alloc_tile_pool(name="wo", bufs=1, side="left")
        p_ps = tc.alloc_tile_pool(name="ps", bufs=6, space="PSUM")
        p_pss = tc.alloc_tile_pool(name="pss", bufs=2, space="PSUM")
        p_v = tc.alloc_tile_pool(name="v", bufs=1, side="right")
        p_xs = tc.alloc_tile_pool(name="xs", bufs=16, side="right")
        p_vs = tc.alloc_tile_pool(name="vs", bufs=16, side="right")
        p_kqt = tc.alloc_tile_pool(name="kqt", bufs=1, side="left")
        p_wa = tc.alloc_tile_pool(name="wa", bufs=1, side="left")

        p_dram = tc.alloc_tile_pool(name="dram", bufs=1, space="DRAM")

        dma = nc.sync.dma_start

        recip_sb = p_misc.tile([P, SQ], F32)

        # Each core projects only its half of the keys; pair-wise AllGather
        # ({2b, 2b+1} share batch b; rank order = k order) fills the rest.
        # The first collective pays a large one-time comm-init cost, so a
        # 128-byte warmup gather is issued immediately and initializes the
        # channels while phase A computes.
        CC_GROUPS = [[0, 1], [2, 3], [4, 5], [6, 7]]
        warm_in = p_dram.tile([1, 64], BF16)
        warm_out = p_dram.tile([2, 64], BF16)
        nc.gpsimd.dma_start(out=warm_in[:], in_=kt_d[0:1, 0:64])
        nc.gpsimd.collective_compute(
            "AllGather",
            mybir.AluOpType.bypass,
            replica_groups=CC_GROUPS,
            ins=[warm_in.opt()],
            outs=[warm_out.opt()],
        )
        cc_kin = p_dram.tile([D, SH], BF16)
        cc_kout = p_dram.tile([2 * D, SH], BF16)
        cc_vin = p_dram.tile([SH, D], BF16)
        cc_vout = p_dram.tile([2 * SH, D], BF16)

        # One DMA per d-block so loads spread across HW queues and each
        # matmul depends only on its own 256KB slice; weights are emitted
        # just before the phase that consumes them so the first matmul
        # isn't queued behind 8MB of unrelated weight traffic.
        def load_w(pool, dram, name):
            t = pool.tile([P, DB, D], BF16, name=name)
            src = dram.rearrange("(a p) e -> p a e", p=P)
            for a in range(DB):
                dma(out=t[:, a, :], in_=src[:, a, :])
            return t

        wkt_sb = load_w(p_wa, wkt_d, "wkt_sb")
        bk_sb = p_misc.tile([P, EB], F32)
        dma(out=bk_sb[:], in_=bk_d[:])
        bq_sb = p_misc.tile([P, EB], F32)
        dma(out=bq_sb[:], in_=bq_d[:])

        kt_sb = p_kqt.tile([P, EB, S], BF16)  # K.T: [e_p, e_blk, k]
        qt_sb = p_kqt.tile([P, EB, SQ], BF16)  # Q.T: [e_p, e_blk, q]
        v_sb = p_v.tile([P, KB, D], BF16)  # V:   [k_p, k_blk, e]

        # ---- Phase A: projections ----
        # Q.T and K.T: out[e, s] = sum_d W.T[d, e] (stationary) @ _X.T[d, s]
        def kq_proj(proj_w, proj_in, proj_out, proj_b, nchunk, sc0=0):
            for sc in range(sc0, sc0 + nchunk):
                xtt = []
                for d in range(DB):
                    t = p_xs.tile([P, CH], BF16, tag="xtt", name="xtt")
                    dma(out=t[:], in_=proj_in[d * P : (d + 1) * P, sc * CH : (sc + 1) * CH])
                    xtt.append(t)
                for eb in range(EB):
                    ps = p_ps.tile([P, CH], F32, tag="ps", name="ps")
                    for d in range(DB):
                        nc.tensor.matmul(
                            ps[:],
                            proj_w[:, d, eb * P : (eb + 1) * P],
                            xtt[d][:],
                            start=(d == 0),
                            stop=(d == DB - 1),
                        )
                    # DVE, not ACT: ~3x faster per copy-out, frees the psum
                    # slot sooner, and keeps ScalarE clear for phase B's exp
                    nc.vector.tensor_scalar_add(
                        proj_out[:, eb, sc * CH : (sc + 1) * CH],
                        ps[:],
                        proj_b[:, eb : eb + 1],
                    )

        # K.T own half into the low half of kt_sb (staging); the gather-back
        # below overwrites all of kt_sb with both halves in global k order.
        kq_proj(wkt_sb, kt_d, kt_sb, bk_sb, SH // CH)
        for eb in range(EB):
            dma(out=cc_kin[eb * P : (eb + 1) * P, :], in_=kt_sb[:, eb, 0:SH])
        nc.gpsimd.collective_compute(
            "AllGather",
            mybir.AluOpType.bypass,
            replica_groups=CC_GROUPS,
            ins=[cc_kin.opt()],
            outs=[cc_kout.opt()],
        )
        for r in range(2):
            for eb in range(EB):
                dma(
                    out=kt_sb[:, eb, r * SH : (r + 1) * SH],
                    in_=cc_kout[r * D + eb * P : r * D + (eb + 1) * P, :],
                )

        wqt_sb = load_w(p_wa, wqt_d, "wqt_sb")
        kq_proj(wqt_sb, qt_d, qt_sb, bq_sb, QC)

        wvt_sb = load_w(p_wa, wvt_d, "wvt_sb")
        bvb_sb = p_misc.tile([P, D], F32)
        dma(out=bvb_sb[:], in_=bvb_d[:])

        # V natural: out[k, e] = sum_d _V.T[d, k] (stationary) @ Wv.T[d, e]
        for kb in range(SH // P):
            vtt = []
            for d in range(DB):
                t = p_vs.tile([P, P], BF16, tag="vtt", name="vtt")
                dma(out=t[:], in_=vt_d[d * P : (d + 1) * P, kb * P : (kb + 1) * P])
                vtt.append(t)
            pse = [
                p_ps.tile([P, CH], F32, tag="ps", name="ps") for _ in range(FC)
            ]
            for d in range(DB):
                for eh in range(FC):
                    nc.tensor.matmul(
                        pse[eh][:],
                        vtt[d][:],
                        wvt_sb[:, d, eh * CH : (eh + 1) * CH],
                        start=(d == 0),
                        stop=(d == DB - 1),
                    )
            for eh in range(FC):
                nc.vector.tensor_add(
                    v_sb[:, kb, eh * CH : (eh + 1) * CH],
                    pse[eh][:],
                    bvb_sb[:, eh * CH : (eh + 1) * CH],
                )

        # gather V halves (own half staged in v_sb[:, 0:8, :])
        for kb in range(SH // P):
            dma(out=cc_vin[kb * P : (kb + 1) * P, :], in_=v_sb[:, kb, :])
        nc.gpsimd.collective_compute(
            "AllGather",
            mybir.AluOpType.bypass,
            replica_groups=CC_GROUPS,
            ins=[cc_vin.opt()],
            outs=[cc_vout.opt()],
        )
        for kb in range(KB):
            dma(out=v_sb[:, kb, :], in_=cc_vout[kb * P : (kb + 1) * P, :])

        ones_sb = p_misc.tile([P, P], BF16)
        nc.vector.memset(ones_sb[:], 1.0)
        wot_sb = load_w(p_wo, wot_d, "wot_sb")
        bob_sb = p_misc.tile([P, D], F32)
        dma(out=bob_sb[:], in_=bob_d[:])

        p_wa.release()
        p_es = tc.alloc_tile_pool(name="es", bufs=1, side="right")
        es_sb = p_es.tile([P, KB, SQ], BF16)  # exp(scores): [k_p, k_blk, q]
        s_ps = [
            p_pss.tile([P, CH], F32, tag="sps", name="s_ps") for _ in range(QC)
        ]

        # ---- Phase B: scores[k, q] = K.T' @ Q.T, exp, and key-sums ----
        for kb in range(KB):
            psq = [
                p_ps.tile([P, CH], F32, tag="ps", name="ps") for _ in range(QC)
            ]
            for eb in range(EB):
                for qc in range(QC):
                    nc.tensor.matmul(
                        psq[qc][:],
                        kt_sb[:, eb, kb * P : (kb + 1) * P],
                        qt_sb[:, eb, qc * CH : (qc + 1) * CH],
                        start=(eb == 0),
                        stop=(eb == EB - 1),
                    )
            for qc in range(QC):
                nc.scalar.activation(
                    es_sb[:, kb, qc * CH : (qc + 1) * CH],
                    psq[qc][:],
                    AF.Exp,
                    scale=float(SCALE),
                )
                # sum_k es[k, q], broadcast to every partition row
                nc.tensor.matmul(
                    s_ps[qc][:],
                    ones_sb[:],
                    es_sb[:, kb, qc * CH : (qc + 1) * CH],
                    start=(kb == 0),
                    stop=(kb == KB - 1),
                )
        for qc in range(QC):
            nc.vector.reciprocal(
                recip_sb[:, qc * CH : (qc + 1) * CH], s_ps[qc][:]
            )

        p_kqt.release()
        p_vtu = tc.alloc_tile_pool(name="vtu", bufs=1, side="left")
        vtu_sb = p_vtu.tile([P, EB, SQ], BF16)  # normalized V_.T: [e_p, e_blk, q]

        # ---- Phase C: V_.T[e, q] = (sum_k V[k, e] es[k, q]) * recip[q] ----
        for eb in range(EB):
            psq = [
                p_ps.tile([P, CH], F32, tag="ps", name="ps") for _ in range(QC)
            ]
            for kb in range(KB):
                for qc in range(QC):
                    nc.tensor.matmul(
                        psq[qc][:],
                        v_sb[:, kb, eb * P : (eb + 1) * P],
                        es_sb[:, kb, qc * CH : (qc + 1) * CH],
                        start=(kb == 0),
                        stop=(kb == KB - 1),
                    )
            for qc in range(QC):
                nc.vector.tensor_mul(
                    vtu_sb[:, eb, qc * CH : (qc + 1) * CH],
                    psq[qc][:],
                    recip_sb[:, qc * CH : (qc + 1) * CH],
                )

        p_o = tc.alloc_tile_pool(name="o", bufs=3, side="left")

        # ---- Phase D: O[q, f] = V_.T' @ Wo.T + bo ----
        for qb in range(QB):
            ot = p_o.tile([P, D], F32, tag="ot", name="ot")
            for fc in range(FC):
                ps = p_ps.tile([P, CH], F32, tag="ps", name="ps")
                for eb in range(EB):
                    nc.tensor.matmul(
                        ps[:],
                        vtu_sb[:, eb, qb * P : (qb + 1) * P],
                        wot_sb[:, eb, fc * CH : (fc + 1) * CH],
                        start=(eb == 0),
                        stop=(eb == EB - 1),
                    )
                nc.vector.tensor_add(
                    ot[:, fc * CH : (fc + 1) * CH],
                    ps[:],
                    bob_sb[:, fc * CH : (fc + 1) * CH],
                )
            # per-chunk stores so the first half ships while the second
            # half's add is still running
            for fc in range(FC):
                dma(
                    out=o_d[qb * P : (qb + 1) * P, fc * CH : (fc + 1) * CH],
                    in_=ot[:, fc * CH : (fc + 1) * CH],
                )

        p_es.release()
        p_vs.release()
        p_xs.release()
        p_v.release()
        p_o.release()
        p_vtu.release()
        p_wo.release()
        p_misc.release()
        p_dram.release()
        p_pss.release()
        p_ps.release()

    nc.finalize()
    return nc


def get_nc() -> bass.Bass:
    global _NC_CACHE
    if _NC_CACHE is None:
        _NC_CACHE = _build_nc()
    return _NC_CACHE


def make_in_maps(inputs: dict) -> list[dict]:
    _K = np.asarray(inputs["_K"], dtype=np.float32)
    _V = np.asarray(inputs["_V"], dtype=np.float32)
    _Q = np.asarray(inputs["_Q"], dtype=np.float32)

    shared = {
        "wkt": np.ascontiguousarray(
            np.asarray(inputs["Wk"], np.float32).T.astype(NPBF16)
        ),
        "wqt": np.ascontiguousarray(
            np.asarray(inputs["Wq"], np.float32).T.astype(NPBF16)
        ),
        "wvt": np.ascontiguousarray(
            np.asarray(inputs["Wv"], np.float32).T.astype(NPBF16)
        ),
        "wot": np.ascontiguousarray(
            np.asarray(inputs["Wo"], np.float32).T.astype(NPBF16)
        ),
        "bk": np.ascontiguousarray(
            np.asarray(inputs["bk"], np.float32).reshape(EB, P).T
        ),
        "bq": np.ascontiguousarray(
            np.asarray(inputs["bq"], np.float32).reshape(EB, P).T
        ),
        "bvb": np.ascontiguousarray(
            np.broadcast_to(np.asarray(inputs["bv"], np.float32), (P, D))
        ),
        "bob": np.ascontiguousarray(
            np.broadcast_to(np.asarray(inputs["bo"], np.float32), (P, D))
        ),
    }

    in_maps = []
    for c in range(8):
        b, h = divmod(c, 2)
        # Each core projects its own key half (h picks it: pair rank order
        # matches k order) and its own query half.
        kt = np.ascontiguousarray(
            _K[b, h * SH : (h + 1) * SH, :].T.astype(NPBF16)
        )
        vt = np.ascontiguousarray(
            _V[b, h * SH : (h + 1) * SH, :].T.astype(NPBF16)
        )
        qt = np.ascontiguousarray(
            _Q[b, h * SQ : (h + 1) * SQ, :].T.astype(NPBF16)
        )
        in_maps.append({"kt": kt, "vt": vt, "qt": qt, **shared})
    return in_maps


def kernel(**inputs) -> np.ndarray:
    global LAST_EXEC_NS
    nc = get_nc()
    in_maps = make_in_maps(inputs)
    kwargs = {}
    if TRACE and TRACE_ALL_CORES:
        kwargs["trace_cores"] = list(range(8))
    res = run_bass_kernel_spmd(
        nc, in_maps, core_ids=list(range(8)), trace=TRACE, **kwargs
    )
    LAST_EXEC_NS = res.exec_time_ns

    out = np.empty((B, S, D), dtype=np.float32)
    for c in range(8):
        b, h = divmod(c, 2)
        out[b, h * SQ : (h + 1) * SQ, :] = res.results[c]["o"]
    return out

